# revision 1
# baseline (speedup 1.0000x reference)
"""GAT (2-layer, PyG-style) on 8 Trainium2 NeuronCores.

Strategy: destination-node sharding (graph parallel). Each core owns a
contiguous range of 6272 destination nodes and all edges pointing into
them (sorted by dst). Source-node features are fetched on-device with
batched indexed gathers (dma_gather) from a per-core *rotated* copy of
the node table, so that a core's own dst rows always sit at small row
indices (dma_gather indices are int16, hence also the A/B table-half
split for the random source indices).

Layer-1 messages are computed on the fly: gather x[src] (bf16, 256 B
rows), then h = x@W1 / e_src = x@w_src / e_dst = x@w_dst as PE matmuls
per 128-edge chunk; segment softmax + scatter-add are done with a
one-hot matmul (S_T^T @ V) accumulated in PSUM per 128-dst-node tile.
The tile tail normalizes by the softmax denominator, applies ReLU and
immediately computes the layer-2 node table row [h2 | e2_src | e2_dst]
via W2. A second launch runs the (structurally identical, 1-head)
layer-2 attention over the host-assembled h2 table and finishes with
log_softmax.
"""

import numpy as np
import ml_dtypes
from contextlib import ExitStack

import concourse.bass as bass
import concourse.mybir as mybir
import concourse.tile as tile
from concourse import bacc
from concourse.bass_utils import run_bass_kernel_spmd

F32 = mybir.dt.float32
BF16 = mybir.dt.bfloat16
I16 = mybir.dt.int16
AF = mybir.ActivationFunctionType
OP = mybir.AluOpType

N = 50000
E = 500000
IN = 128
HID = 64
HEADS = 8
OUT = 40
NEG = 0.2
NCORE = 8
P = 128
TILES = 49
SHARD = TILES * P          # 6272
NPAD = NCORE * SHARD       # 50176
SPLIT = 32768              # int16 table-half split
NB = NPAD - SPLIT          # 17408

_bf16 = ml_dtypes.bfloat16

_CACHE = {}

# Gather sizing: one dma_gather of n indices emits n/16+2 descriptors per
# SDMA engine; with single_packet=True a packet holds at most 64
# descriptors, so calls >992 indices wedge the device. 896 indices
# (58 descs) stays under the cap; multi-call concurrency at this size is
# throttled safely by ucode (verified on HW).
GCAP = 896
GSP = True  # single_packet


def _gather(nc, out3, in_ap, idx_sb, col0, n, elem):
    """dma_gather split into <=GCAP-index calls. out3: [P, 1|chunks, *]
    destination AP covering exactly n indices starting at its origin."""
    done = 0
    while done < n:
        take = min(GCAP, n - done)
        if out3.ndim == 3 and out3.shape[2] != elem:  # transpose=True layout
            o = out3[:, :, done : done + take]
            tr = True
        else:  # [P, chunks, elem] layout
            o = out3[:, done // P : (done + take) // P, :]
            tr = False
        nc.gpsimd.dma_gather(
            out_ap=o,
            in_ap=in_ap,
            idxs_ap=idx_sb[:, col0 + done // 16 : col0 + (done + take) // 16],
            num_idxs=take,
            num_idxs_reg=take,
            elem_size=elem,
            transpose=tr,
            single_packet=GSP,
        )
        done += take


def _wrap16(v):
    """dma_gather index layout: idx[p, j] = stream[j*16 + p%16], replicated
    to 128 partitions."""
    assert len(v) % 16 == 0
    w = v.reshape(-1, 16).T.astype(np.int16)   # [16, n/16]
    return np.tile(w, (8, 1))                  # [128, n/16]


def _prep_edges(edge_index):
    """Bucket edges (+self-loops) by dst core, sort by dst, split by
    src-table half, pad to SPMD-uniform per-tile sizes.

    Returns per-tile padded sizes EA/EB (shared by all cores) and the
    per-core index/metadata streams."""
    src = np.concatenate([np.asarray(edge_index[0]), np.arange(N)]).astype(np.int64)
    dst = np.concatenate([np.asarray(edge_index[1]), np.arange(N)]).astype(np.int64)
    core = dst // SHARD

    pc = []  # per-core (tile -> (a_idx, b_idx, dloc_a, dloc_b))
    nA = np.zeros((NCORE, TILES), np.int64)
    nB = np.zeros((NCORE, TILES), np.int64)
    for c in range(NCORE):
        m = core == c
        s = src[m]
        dl = dst[m] - c * SHARD
        o = np.argsort(dl, kind="stable")
        s = s[o]
        dl = dl[o]
        sr = (s - c * SHARD) % NPAD  # rotated source row
        bounds = np.searchsorted(dl, np.arange(TILES + 1) * P)
        tl = []
        for t in range(TILES):
            lo, hi = bounds[t], bounds[t + 1]
            srt, dlt = sr[lo:hi], dl[lo:hi] % P
            ma = srt < SPLIT
            tl.append((srt[ma], srt[~ma] - SPLIT, dlt[ma], dlt[~ma]))
            nA[c, t] = ma.sum()
            nB[c, t] = (~ma).sum()
        pc.append(tl)

    rup = lambda n: int(-(-n // P) * P)
    EA = [rup(nA[:, t].max()) for t in range(TILES)]
    EB = [rup(nB[:, t].max()) for t in range(TILES)]

    streams = []
    for c in range(NCORE):
        ia, ib, idd, dlc = [], [], [], []
        for t in range(TILES):
            a, b, da, db = pc[c][t]
            pa = np.zeros(EA[t], np.int64)
            pa[: len(a)] = a
            pb = np.zeros(EB[t], np.int64)
            pb[: len(b)] = b
            ia.append(pa)
            ib.append(pb)
            # dst-row gather stream + dst-local values, in slot order [A|B]
            dr = np.zeros(EA[t] + EB[t], np.int64)
            dv = np.full(EA[t] + EB[t], 200.0, np.float32)
            dr[: len(a)] = da + t * P
            dv[: len(a)] = da
            dr[EA[t] : EA[t] + len(b)] = db + t * P
            dv[EA[t] : EA[t] + len(b)] = db
            idd.append(dr)
            dlc.append(dv)
        ept = np.concatenate(idd)
        # L2 combined stream: per tile [srcA-padded | dst] (both read htA)
        iad = [np.concatenate([ia[t], idd[t]]) for t in range(TILES)]
        streams.append(
            dict(
                idxA=_wrap16(np.concatenate(ia)),
                idxB=_wrap16(np.concatenate(ib)),
                idxD=_wrap16(ept),
                idxAD=_wrap16(np.concatenate(iad)),
                dloc=np.concatenate(dlc).reshape(-1, P).T.copy(),  # [128, nchunks]
            )
        )
    return EA, EB, streams


def _build_l1(EA, EB):
    colsA = sum(EA) // 16
    colsB = sum(EB) // 16
    EPT = [a + b for a, b in zip(EA, EB)]
    colsD = sum(EPT) // 16
    nch_tot = sum(EPT) // P

    nc = bacc.Bacc("TRN2", target_bir_lowering=False, debug=False, num_devices=NCORE)
    xtA = nc.dram_tensor("xtA", [SPLIT, IN], BF16, kind="ExternalInput")
    xtB = nc.dram_tensor("xtB", [NB, IN], BF16, kind="ExternalInput")
    idxA = nc.dram_tensor("idxA", [P, max(colsA, 1)], I16, kind="ExternalInput")
    idxB = nc.dram_tensor("idxB", [P, max(colsB, 1)], I16, kind="ExternalInput")
    idxD = nc.dram_tensor("idxD", [P, colsD], I16, kind="ExternalInput")
    dloc = nc.dram_tensor("dloc", [P, nch_tot], F32, kind="ExternalInput")
    w1 = nc.dram_tensor("w1", [P, HEADS * HID], BF16, kind="ExternalInput")
    wsd = nc.dram_tensor("wsd", [P, 2 * HEADS], BF16, kind="ExternalInput")
    w2c = nc.dram_tensor("w2c", [P, 4 * 42], BF16, kind="ExternalInput")
    iot = nc.dram_tensor("iot", [P, P], BF16, kind="ExternalInput")
    idn = nc.dram_tensor("idn", [P, P], BF16, kind="ExternalInput")
    h2row = nc.dram_tensor("h2row", [SHARD, 64], F32, kind="ExternalOutput")

    with tile.TileContext(nc) as tc, ExitStack() as ctx:
        cp = ctx.enter_context(tc.tile_pool(name="const", bufs=1))
        gp = ctx.enter_context(tc.tile_pool(name="gath", bufs=12))
        sp = ctx.enter_context(tc.tile_pool(name="small", bufs=12))
        vp = ctx.enter_context(tc.tile_pool(name="vals", bufs=8))
        rp = ctx.enter_context(tc.tile_pool(name="tail", bufs=3))
        ph = ctx.enter_context(tc.tile_pool(name="ph", bufs=2, space="PSUM"))
        pe = ctx.enter_context(tc.tile_pool(name="pe", bufs=2, space="PSUM"))
        po = ctx.enter_context(tc.tile_pool(name="po", bufs=1, space="PSUM"))
        pz = ctx.enter_context(tc.tile_pool(name="pz", bufs=1, space="PSUM"))
        pt = ctx.enter_context(tc.tile_pool(name="pt", bufs=1, space="PSUM"))
        p2 = ctx.enter_context(tc.tile_pool(name="p2", bufs=1, space="PSUM"))

        w1sb = cp.tile([P, HEADS * HID], BF16)
        nc.sync.dma_start(w1sb[:], w1.ap())
        wsdsb = cp.tile([P, 2 * HEADS], BF16)
        nc.sync.dma_start(wsdsb[:], wsd.ap())
        w2csb = cp.tile([P, 4 * 42], BF16)
        nc.sync.dma_start(w2csb[:], w2c.ap())
        iosb = cp.tile([P, P], BF16)
        nc.sync.dma_start(iosb[:], iot.ap())
        idsb = cp.tile([P, P], BF16)
        nc.sync.dma_start(idsb[:], idn.ap())
        iAsb = cp.tile([P, max(colsA, 1)], I16)
        nc.sync.dma_start(iAsb[:], idxA.ap())
        iBsb = cp.tile([P, max(colsB, 1)], I16)
        nc.sync.dma_start(iBsb[:], idxB.ap())
        iDsb = cp.tile([P, colsD], I16)
        nc.sync.dma_start(iDsb[:], idxD.ap())
        dlsb = cp.tile([P, nch_tot], F32)
        nc.sync.dma_start(dlsb[:], dloc.ap())

        oa = ob = od = co = 0
        for t in range(TILES):
            ea, eb = EA[t], EB[t]
            ept = ea + eb
            nchk = ept // P
            xg = gp.tile([P, 1, ept], BF16, tag="xg")
            if ea:
                _gather(nc, xg[:, :, 0:ea], xtA.ap(), iAsb, oa, ea, IN)
            if eb:
                _gather(nc, xg[:, :, ea:ept], xtB.ap(), iBsb, ob, eb, IN)
            xd = gp.tile([P, 1, ept], BF16, tag="xd")
            _gather(nc, xd[:, :, 0:ept], xtA.ap(), iDsb, od, ept, IN)

            o1ps = po.tile([P, HEADS * HID], F32, tag="o1")
            zps = pz.tile([P, HEADS], F32, tag="z")
            for k in range(nchk):
                ls = xg[:, 0, k * P : (k + 1) * P]
                ld = xd[:, 0, k * P : (k + 1) * P]
                hps = ph.tile([P, HEADS * HID], F32, tag="h")
                nc.tensor.matmul(hps[:], lhsT=ls, rhs=w1sb[:], start=True, stop=True)
                eps = pe.tile([P, HEADS], F32, tag="e")
                nc.tensor.matmul(
                    eps[:], lhsT=ls, rhs=wsdsb[:, 0:HEADS],
                    start=True, stop=False,
                )
                nc.tensor.matmul(
                    eps[:], lhsT=ld,
                    rhs=wsdsb[:, HEADS : 2 * HEADS], start=False, stop=True,
                )
                st = sp.tile([P, P], BF16, tag="st")
                nc.vector.tensor_scalar(
                    out=st[:], in0=iosb[:], scalar1=dlsb[:, co + k : co + k + 1],
                    scalar2=None, op0=OP.is_equal,
                )
                ll = sp.tile([P, HEADS], F32, tag="ll")
                nc.vector.tensor_scalar(
                    out=ll[:], in0=eps[:], scalar1=NEG, scalar2=None, op0=OP.mult
                )
                lr = sp.tile([P, HEADS], F32, tag="lr")
                nc.vector.tensor_tensor(out=lr[:], in0=eps[:], in1=ll[:], op=OP.max)
                p32 = sp.tile([P, HEADS], F32, tag="p32")
                nc.scalar.activation(out=p32[:], in_=lr[:], func=AF.Exp)
                pbf = sp.tile([P, HEADS], BF16, tag="pbf")
                nc.vector.tensor_copy(out=pbf[:], in_=p32[:])
                vt = vp.tile([P, HEADS, HID], BF16, tag="vt")
                nc.vector.tensor_tensor(
                    out=vt[:],
                    in0=hps[:].rearrange("p (h c) -> p h c", c=HID),
                    in1=p32[:].unsqueeze(2).to_broadcast([P, HEADS, HID]),
                    op=OP.mult,
                )
                nc.tensor.matmul(
                    o1ps[:], lhsT=st[:], rhs=vt[:].rearrange("p h c -> p (h c)"),
                    start=(k == 0), stop=(k == nchk - 1),
                )
                nc.tensor.matmul(
                    zps[:], lhsT=st[:], rhs=pbf[:],
                    start=(k == 0), stop=(k == nchk - 1),
                )

            zr = sp.tile([P, HEADS], F32, tag="zr")
            nc.vector.reciprocal(zr[:], zps[:])
            r1 = rp.tile([P, HEADS * HID], BF16, tag="r1")
            for h in range(HEADS):
                nc.scalar.activation(
                    out=r1[:, h * HID : (h + 1) * HID],
                    in_=o1ps[:, h * HID : (h + 1) * HID],
                    func=AF.Relu,
                    scale=zr[:, h : h + 1],
                )
            h2ps = p2.tile([P, 48], F32, tag="h2")
            for j in range(4):
                tp = pt.tile([P, P], BF16, tag="tp")
                nc.tensor.transpose(tp[:], r1[:, j * P : (j + 1) * P], idsb[:])
                tsb = rp.tile([P, P], BF16, tag="tsb")
                nc.scalar.activation(out=tsb[:], in_=tp[:], func=AF.Copy)
                nc.tensor.matmul(
                    h2ps[:, 0:42], lhsT=tsb[:], rhs=w2csb[:, j * 42 : (j + 1) * 42],
                    start=(j == 0), stop=(j == 3),
                )
            o1 = rp.tile([P, 64], F32, tag="o1s")
            nc.scalar.activation(out=o1[:, 0:42], in_=h2ps[:, 0:42], func=AF.Copy)
            nc.vector.memset(o1[:, 42:64], 0.0)
            nc.sync.dma_start(h2row.ap()[t * P : (t + 1) * P, :], o1[:])

            oa += ea // 16
            ob += eb // 16
            od += ept // 16
            co += nchk
    nc.compile()
    return nc


def _build_l2(EA, EB):
    colsA = sum(EA) // 16
    colsB = sum(EB) // 16
    EPT = [a + b for a, b in zip(EA, EB)]
    colsD = sum(EPT) // 16
    nch_tot = sum(EPT) // P

    colsAD = colsA + colsD
    nc = bacc.Bacc("TRN2", target_bir_lowering=False, debug=False, num_devices=NCORE)
    htA = nc.dram_tensor("htA", [SPLIT, 64], F32, kind="ExternalInput")
    htB = nc.dram_tensor("htB", [NB, 64], F32, kind="ExternalInput")
    idxAD = nc.dram_tensor("idxAD", [P, colsAD], I16, kind="ExternalInput")
    idxB = nc.dram_tensor("idxB", [P, max(colsB, 1)], I16, kind="ExternalInput")
    dloc = nc.dram_tensor("dloc", [P, nch_tot], F32, kind="ExternalInput")
    iot = nc.dram_tensor("iot", [P, P], BF16, kind="ExternalInput")
    out2 = nc.dram_tensor("out2", [SHARD, OUT], F32, kind="ExternalOutput")

    with tile.TileContext(nc) as tc, ExitStack() as ctx:
        cp = ctx.enter_context(tc.tile_pool(name="const", bufs=1))
        gp = ctx.enter_context(tc.tile_pool(name="gath", bufs=3))
        sp = ctx.enter_context(tc.tile_pool(name="small", bufs=4))
        rp = ctx.enter_context(tc.tile_pool(name="tail", bufs=2))
        po = ctx.enter_context(tc.tile_pool(name="po", bufs=2, space="PSUM"))
        pz = ctx.enter_context(tc.tile_pool(name="pz", bufs=2, space="PSUM"))

        iosb = cp.tile([P, P], BF16)
        nc.sync.dma_start(iosb[:], iot.ap())
        iADsb = cp.tile([P, colsAD], I16)
        nc.sync.dma_start(iADsb[:], idxAD.ap())
        iBsb = cp.tile([P, max(colsB, 1)], I16)
        nc.sync.dma_start(iBsb[:], idxB.ap())
        dlsb = cp.tile([P, nch_tot], F32)
        nc.sync.dma_start(dlsb[:], dloc.ap())

        oad = ob = co = 0
        for t in range(TILES):
            ea, eb = EA[t], EB[t]
            ept = ea + eb
            nchk = ept // P
            # tile layout: [srcA chunks | dst chunks | srcB chunks]
            gad = gp.tile([P, nchk + ept // P, 64], F32, tag="g2")
            _gather(nc, gad[:, 0 : (ea + ept) // P, :], htA.ap(), iADsb, oad,
                    ea + ept, 64)
            if eb:
                _gather(nc, gad[:, (ea + ept) // P :, :], htB.ap(), iBsb, ob,
                        eb, 64)

            def _src(k, ea=ea, ept=ept, gad=gad):
                return gad[:, k, :] if k < ea // P else gad[:, ept // P + k, :]

            def _dst(k, ea=ea, gad=gad):
                return gad[:, ea // P + k, :]

            o2ps = po.tile([P, 48], F32, tag="o2")
            z2ps = pz.tile([P, 8], F32, tag="z2")
            for k in range(nchk):
                st = sp.tile([P, P], BF16, tag="st")
                nc.vector.tensor_scalar(
                    out=st[:], in0=iosb[:], scalar1=dlsb[:, co + k : co + k + 1],
                    scalar2=None, op0=OP.is_equal,
                )
                lg = sp.tile([P, 1], F32, tag="lg")
                nc.vector.tensor_tensor(
                    out=lg[:], in0=_src(k)[:, 40:41], in1=_dst(k)[:, 41:42],
                    op=OP.add,
                )
                ll = sp.tile([P, 1], F32, tag="ll")
                nc.vector.tensor_scalar(
                    out=ll[:], in0=lg[:], scalar1=NEG, scalar2=None, op0=OP.mult
                )
                lr = sp.tile([P, 1], F32, tag="lr")
                nc.vector.tensor_tensor(out=lr[:], in0=lg[:], in1=ll[:], op=OP.max)
                p32 = sp.tile([P, 1], F32, tag="p32")
                nc.scalar.activation(out=p32[:], in_=lr[:], func=AF.Exp)
                pbf = sp.tile([P, 1], BF16, tag="pbf")
                nc.vector.tensor_copy(out=pbf[:], in_=p32[:])
                v2 = sp.tile([P, OUT], BF16, tag="v2")
                nc.scalar.activation(
                    out=v2[:], in_=_src(k)[:, 0:OUT], func=AF.Identity,
                    scale=p32[:],
                )
                nc.tensor.matmul(
                    o2ps[:, 0:OUT], lhsT=st[:], rhs=v2[:],
                    start=(k == 0), stop=(k == nchk - 1),
                )
                nc.tensor.matmul(
                    z2ps[:, 0:1], lhsT=st[:], rhs=pbf[:],
                    start=(k == 0), stop=(k == nchk - 1),
                )

            zr = sp.tile([P, 1], F32, tag="zr")
            nc.vector.reciprocal(zr[:], z2ps[:, 0:1])
            av = rp.tile([P, OUT], F32, tag="av")
            nc.vector.tensor_scalar(
                out=av[:], in0=o2ps[:, 0:OUT], scalar1=zr[:], scalar2=None,
                op0=OP.mult,
            )
            mx = sp.tile([P, 1], F32, tag="mx")
            nc.vector.reduce_max(out=mx[:], in_=av[:], axis=mybir.AxisListType.X)
            tm = rp.tile([P, OUT], F32, tag="tm")
            nc.vector.tensor_scalar(
                out=tm[:], in0=av[:], scalar1=mx[:], scalar2=None, op0=OP.subtract
            )
            ex = rp.tile([P, OUT], F32, tag="ex")
            nc.scalar.activation(out=ex[:], in_=tm[:], func=AF.Exp)
            sm = sp.tile([P, 1], F32, tag="sm")
            nc.vector.reduce_sum(out=sm[:], in_=ex[:], axis=mybir.AxisListType.X)
            ls = sp.tile([P, 1], F32, tag="ls")
            nc.scalar.activation(out=ls[:], in_=sm[:], func=AF.Ln)
            fin = rp.tile([P, OUT], F32, tag="fin")
            nc.vector.tensor_scalar(
                out=fin[:], in0=tm[:], scalar1=ls[:], scalar2=None, op0=OP.subtract
            )
            nc.sync.dma_start(out2.ap()[t * P : (t + 1) * P, :], fin[:])

            oad += (ea + ept) // 16
            ob += eb // 16
            co += nchk
    nc.compile()
    return nc


def _prepare(x, edge_index, W1, a1_src, a1_dst, W2, a2_src, a2_dst):
    key = hash(np.asarray(edge_index).tobytes())
    if key in _CACHE:
        return _CACHE[key]
    EA, EB, streams = _prep_edges(edge_index)
    l1 = _build_l1(EA, EB)
    l2 = _build_l2(EA, EB)
    _CACHE.clear()
    _CACHE[key] = (EA, EB, streams, l1, l2)
    return _CACHE[key]


def _host_consts(x, W1, a1_src, a1_dst, W2, a2_src, a2_dst):
    x = np.asarray(x, np.float32)
    W1 = np.asarray(W1, np.float32)
    W2 = np.asarray(W2, np.float32)
    a1_src = np.asarray(a1_src, np.float32)
    a1_dst = np.asarray(a1_dst, np.float32)
    a2_src = np.asarray(a2_src, np.float32).reshape(-1)
    a2_dst = np.asarray(a2_dst, np.float32).reshape(-1)

    xpad = np.zeros((NPAD, IN), np.float32)
    xpad[:N] = x
    W1r = W1.reshape(IN, HEADS, HID)
    wsd = np.concatenate(
        [np.einsum("khc,hc->kh", W1r, a1_src), np.einsum("khc,hc->kh", W1r, a1_dst)],
        axis=1,
    )  # [128, 16]
    wv2s = W2 @ a2_src  # [512]
    wv2d = W2 @ a2_dst
    w2c = np.zeros((P, 4 * 42), np.float32)
    for j in range(4):
        w2c[:, j * 42 : j * 42 + 40] = W2[j * P : (j + 1) * P, :]
        w2c[:, j * 42 + 40] = wv2s[j * P : (j + 1) * P]
        w2c[:, j * 42 + 41] = wv2d[j * P : (j + 1) * P]
    iot = np.tile(np.arange(P, dtype=np.float32), (P, 1)).astype(_bf16)
    idn = np.eye(P, dtype=np.float32)
    return xpad, wsd.astype(_bf16), w2c.astype(_bf16), iot, idn.astype(_bf16), W1.astype(_bf16)


def _run(inputs, trace=False):
    x = inputs["x"]
    edge_index = inputs["edge_index"]
    EA, EB, streams, l1, l2 = _prepare(
        x, edge_index, inputs["W1"], inputs["a1_src"], inputs["a1_dst"],
        inputs["W2"], inputs["a2_src"], inputs["a2_dst"],
    )
    xpad, wsd, w2c, iot, idn, W1bf = _host_consts(
        x, inputs["W1"], inputs["a1_src"], inputs["a1_dst"],
        inputs["W2"], inputs["a2_src"], inputs["a2_dst"],
    )

    in_maps = []
    for c in range(NCORE):
        xr = np.roll(xpad, -c * SHARD, axis=0).astype(_bf16)
        s = streams[c]
        in_maps.append(
            dict(
                xtA=xr[:SPLIT], xtB=xr[SPLIT:],
                idxA=s["idxA"], idxB=s["idxB"], idxD=s["idxD"],
                dloc=np.ascontiguousarray(s["dloc"]),
                w1=W1bf, wsd=wsd, w2c=w2c, iot=iot, idn=idn,
            )
        )
    def _launch(prog, maps):
        try:
            return run_bass_kernel_spmd(prog, maps, list(range(NCORE)), trace=trace)
        except Exception:
            import time as _time
            _time.sleep(5)
            return run_bass_kernel_spmd(prog, maps, list(range(NCORE)), trace=trace)

    r1 = _launch(l1, in_maps)
    h2tab = np.zeros((NPAD, 64), np.float32)
    for c in range(NCORE):
        h2tab[c * SHARD : (c + 1) * SHARD] = r1.results[c]["h2row"]
    h2tab[N:] = 0.0

    in_maps2 = []
    for c in range(NCORE):
        hr = np.roll(h2tab, -c * SHARD, axis=0)
        s = streams[c]
        in_maps2.append(
            dict(
                htA=np.ascontiguousarray(hr[:SPLIT]),
                htB=np.ascontiguousarray(hr[SPLIT:]),
                idxAD=s["idxAD"], idxB=s["idxB"],
                dloc=np.ascontiguousarray(s["dloc"]), iot=iot,
            )
        )
    r2 = _launch(l2, in_maps2)
    out = np.concatenate([r2.results[c]["out2"] for c in range(NCORE)], axis=0)[:N]
    ns = None
    if r1.exec_time_ns is not None and r2.exec_time_ns is not None:
        ns = r1.exec_time_ns + r2.exec_time_ns
    return np.ascontiguousarray(out, dtype=np.float32), ns


def kernel(**inputs) -> np.ndarray:
    out, _ = _run(inputs, trace=False)
    return out



# revision 9
# speedup vs baseline: 1.2533x; 1.2533x over previous
"""GAT (2-layer, PyG-style) on 8 Trainium2 NeuronCores.

Strategy: destination-node sharding (graph parallel). Each core owns a
contiguous range of 6272 destination nodes and all edges pointing into
them (sorted by dst). Source-node features are fetched on-device with
batched indexed gathers (dma_gather) from a per-core *rotated* copy of
the node table, so that a core's own dst rows always sit at small row
indices (dma_gather indices are int16, hence also the A/B table-half
split for the random source indices).

Layer-1 messages are computed on the fly: gather x[src] (bf16, 256 B
rows), then h = x@W1 / e_src = x@w_src / e_dst = x@w_dst as PE matmuls
per 128-edge chunk; segment softmax + scatter-add are done with a
one-hot matmul (S_T^T @ V) accumulated in PSUM per 128-dst-node tile.
The tile tail normalizes by the softmax denominator, applies ReLU and
immediately computes the layer-2 node table row [h2 | e2_src | e2_dst]
via W2. A second launch runs the (structurally identical, 1-head)
layer-2 attention over the host-assembled h2 table and finishes with
log_softmax.
"""

import numpy as np
import ml_dtypes
from contextlib import ExitStack

import concourse.bass as bass
import concourse.mybir as mybir
import concourse.tile as tile
from concourse import bacc
from concourse.bass_utils import run_bass_kernel_spmd

F32 = mybir.dt.float32
BF16 = mybir.dt.bfloat16
I16 = mybir.dt.int16
AF = mybir.ActivationFunctionType
OP = mybir.AluOpType

N = 50000
E = 500000
IN = 128
HID = 64
HEADS = 8
OUT = 40
NEG = 0.2
NCORE = 8
P = 128
TILES = 49
SHARD = TILES * P          # 6272
NPAD = NCORE * SHARD       # 50176
SPLIT = 32768              # int16 table-half split
NB = NPAD - SPLIT          # 17408

_bf16 = ml_dtypes.bfloat16

_CACHE = {}

# Gather sizing: one dma_gather of n indices emits n/16+2 descriptors per
# SDMA engine; with single_packet=True a packet holds at most 64
# descriptors, so calls >992 indices wedge the device. 896 indices
# (58 descs) stays under the cap; multi-call concurrency at this size is
# throttled safely by ucode (verified on HW).
GCAP = 896
GSP = True  # single_packet


def _gather(nc, out3, in_ap, idx_sb, col0, n, elem):
    """dma_gather split into <=GCAP-index calls. out3: [P, 1|chunks, *]
    destination AP covering exactly n indices starting at its origin."""
    done = 0
    while done < n:
        take = min(GCAP, n - done)
        if out3.ndim == 3 and out3.shape[2] != elem:  # transpose=True layout
            o = out3[:, :, done : done + take]
            tr = True
        else:  # [P, chunks, elem] layout
            o = out3[:, done // P : (done + take) // P, :]
            tr = False
        nc.gpsimd.dma_gather(
            out_ap=o,
            in_ap=in_ap,
            idxs_ap=idx_sb[:, col0 + done // 16 : col0 + (done + take) // 16],
            num_idxs=take,
            num_idxs_reg=take,
            elem_size=elem,
            transpose=tr,
            single_packet=GSP,
        )
        done += take


def _wrap16(v):
    """dma_gather index layout: idx[p, j] = stream[j*16 + p%16], replicated
    to 128 partitions."""
    assert len(v) % 16 == 0
    w = v.reshape(-1, 16).T.astype(np.int16)   # [16, n/16]
    return np.tile(w, (8, 1))                  # [128, n/16]


def _prep_edges(edge_index):
    """Bucket edges (+self-loops) by dst core, sort by dst, split by
    src-table half, pad to SPMD-uniform per-tile sizes.

    Returns per-tile padded sizes EA/EB (shared by all cores) and the
    per-core index/metadata streams."""
    src = np.concatenate([np.asarray(edge_index[0]), np.arange(N)]).astype(np.int64)
    dst = np.concatenate([np.asarray(edge_index[1]), np.arange(N)]).astype(np.int64)
    core = dst // SHARD

    pc = []  # per-core (tile -> (a_idx, b_idx, dloc_a, dloc_b))
    nA = np.zeros((NCORE, TILES), np.int64)
    nB = np.zeros((NCORE, TILES), np.int64)
    for c in range(NCORE):
        m = core == c
        s = src[m]
        dl = dst[m] - c * SHARD
        o = np.argsort(dl, kind="stable")
        s = s[o]
        dl = dl[o]
        sr = (s - c * SHARD) % NPAD  # rotated source row
        bounds = np.searchsorted(dl, np.arange(TILES + 1) * P)
        tl = []
        for t in range(TILES):
            lo, hi = bounds[t], bounds[t + 1]
            srt, dlt = sr[lo:hi], dl[lo:hi] % P
            ma = srt < SPLIT
            tl.append((srt[ma], srt[~ma] - SPLIT, dlt[ma], dlt[~ma]))
            nA[c, t] = ma.sum()
            nB[c, t] = (~ma).sum()
        pc.append(tl)

    rup = lambda n: int(-(-n // P) * P)
    EA = [rup(nA[:, t].max()) for t in range(TILES)]
    EB = [rup(nB[:, t].max()) for t in range(TILES)]

    streams = []
    for c in range(NCORE):
        ia, ib, idd, dlc = [], [], [], []
        for t in range(TILES):
            a, b, da, db = pc[c][t]
            pa = np.zeros(EA[t], np.int64)
            pa[: len(a)] = a
            pb = np.zeros(EB[t], np.int64)
            pb[: len(b)] = b
            ia.append(pa)
            ib.append(pb)
            # dst-row gather stream + dst-local values, in slot order [A|B]
            dr = np.zeros(EA[t] + EB[t], np.int64)
            dv = np.full(EA[t] + EB[t], 200.0, np.float32)
            dr[: len(a)] = da + t * P
            dv[: len(a)] = da
            dr[EA[t] : EA[t] + len(b)] = db + t * P
            dv[EA[t] : EA[t] + len(b)] = db
            idd.append(dr)
            dlc.append(dv)
        ept = np.concatenate(idd)
        # L2 combined stream: per tile [srcA-padded | dst] (both read htA)
        iad = [np.concatenate([ia[t], idd[t]]) for t in range(TILES)]
        streams.append(
            dict(
                idxA=_wrap16(np.concatenate(ia)),
                idxB=_wrap16(np.concatenate(ib)),
                idxD=_wrap16(ept),
                idxAD=_wrap16(np.concatenate(iad)),
                dloc=np.concatenate(dlc).reshape(-1, P).T.copy(),  # [128, nchunks]
            )
        )
    return EA, EB, streams


def _build_l1(EA, EB):
    colsA = sum(EA) // 16
    colsB = sum(EB) // 16
    EPT = [a + b for a, b in zip(EA, EB)]
    colsD = sum(EPT) // 16
    nch_tot = sum(EPT) // P

    nc = bacc.Bacc("TRN2", target_bir_lowering=False, debug=False, num_devices=NCORE)
    xtA = nc.dram_tensor("xtA", [SPLIT, IN], BF16, kind="ExternalInput")
    xtB = nc.dram_tensor("xtB", [NB, IN], BF16, kind="ExternalInput")
    idxA = nc.dram_tensor("idxA", [P, max(colsA, 1)], I16, kind="ExternalInput")
    idxB = nc.dram_tensor("idxB", [P, max(colsB, 1)], I16, kind="ExternalInput")
    idxD = nc.dram_tensor("idxD", [P, colsD], I16, kind="ExternalInput")
    dloc = nc.dram_tensor("dloc", [P, nch_tot], F32, kind="ExternalInput")
    w1 = nc.dram_tensor("w1", [P, HEADS * HID], BF16, kind="ExternalInput")
    wsd = nc.dram_tensor("wsd", [P, 2 * HEADS], BF16, kind="ExternalInput")
    w2c = nc.dram_tensor("w2c", [P, 4 * 42], BF16, kind="ExternalInput")
    iot = nc.dram_tensor("iot", [P, P], BF16, kind="ExternalInput")
    idn = nc.dram_tensor("idn", [P, P], BF16, kind="ExternalInput")
    h2row = nc.dram_tensor("h2row", [SHARD, 64], F32, kind="ExternalOutput")

    with tile.TileContext(nc) as tc, ExitStack() as ctx:
        cp = ctx.enter_context(tc.tile_pool(name="const", bufs=1))
        gp = ctx.enter_context(tc.tile_pool(name="gath", bufs=12))
        sp = ctx.enter_context(tc.tile_pool(name="small", bufs=12))
        vp = ctx.enter_context(tc.tile_pool(name="vals", bufs=4))
        rp = ctx.enter_context(tc.tile_pool(name="tail", bufs=3))
        # PSUM budget (8 banks): ph h-pairs 2x2=4, pm eps 1, pt tp 1, po o1 1,
        # pz 1 (z cols 0:8 + h2 cols 64:112 carved from one bank tile)
        ph = ctx.enter_context(tc.tile_pool(name="ph", bufs=2, space="PSUM"))
        pm = ctx.enter_context(tc.tile_pool(name="pm", bufs=1, space="PSUM"))
        pt = ctx.enter_context(tc.tile_pool(name="pt", bufs=1, space="PSUM"))
        po = ctx.enter_context(tc.tile_pool(name="po", bufs=1, space="PSUM"))
        pz = ctx.enter_context(tc.tile_pool(name="pz", bufs=1, space="PSUM"))

        w1sb = cp.tile([P, HEADS * HID], BF16)
        nc.sync.dma_start(w1sb[:], w1.ap())
        wsdsb = cp.tile([P, 2 * HEADS], BF16)
        nc.sync.dma_start(wsdsb[:], wsd.ap())
        w2csb = cp.tile([P, 4 * 42], BF16)
        nc.sync.dma_start(w2csb[:], w2c.ap())
        iosb = cp.tile([P, P], BF16)
        nc.sync.dma_start(iosb[:], iot.ap())
        idsb = cp.tile([P, P], BF16)
        nc.sync.dma_start(idsb[:], idn.ap())
        iAsb = cp.tile([P, max(colsA, 1)], I16)
        nc.sync.dma_start(iAsb[:], idxA.ap())
        iBsb = cp.tile([P, max(colsB, 1)], I16)
        nc.sync.dma_start(iBsb[:], idxB.ap())
        iDsb = cp.tile([P, colsD], I16)
        nc.sync.dma_start(iDsb[:], idxD.ap())
        dlsb = cp.tile([P, nch_tot], F32)
        nc.sync.dma_start(dlsb[:], dloc.ap())

        oa = ob = od = co = 0
        for t in range(TILES):
            ea, eb = EA[t], EB[t]
            ept = ea + eb
            nchk = ept // P
            xg = gp.tile([P, 1, ept], BF16, tag="xg")
            if ea:
                _gather(nc, xg[:, :, 0:ea], xtA.ap(), iAsb, oa, ea, IN)
            if eb:
                _gather(nc, xg[:, :, ea:ept], xtB.ap(), iBsb, ob, eb, IN)
            xd = gp.tile([P, 1, ept], BF16, tag="xd")
            _gather(nc, xd[:, :, 0:ept], xtA.ap(), iDsb, od, ept, IN)

            o1ps = po.tile([P, HEADS * HID], F32, tag="o1")
            zz = pz.tile([P, 512], F32, tag="zz")
            zps = zz[:, 0:HEADS]
            G = 4
            for g in range(0, nchk, G):
                gsz = min(G, nchk - g)
                # logits for the group: one PSUM tile, one 8-col region/chunk
                eps4 = pm.tile([P, 8 * gsz], F32, tag="eps")
                for j in range(gsz):
                    ls = xg[:, 0, (g + j) * P : (g + j + 1) * P]
                    ld = xd[:, 0, (g + j) * P : (g + j + 1) * P]
                    nc.tensor.matmul(
                        eps4[:, j * 8 : (j + 1) * 8], lhsT=ls,
                        rhs=wsdsb[:, 0:HEADS], start=True, stop=False,
                    )
                    nc.tensor.matmul(
                        eps4[:, j * 8 : (j + 1) * 8], lhsT=ld,
                        rhs=wsdsb[:, HEADS : 2 * HEADS], start=False, stop=True,
                    )
                # h for the group: chunk pairs share a 2-bank PSUM tile
                hts = []
                for p0 in range(0, gsz, 2):
                    psz = min(2, gsz - p0)
                    hp = ph.tile([P, psz * 512], F32, tag="h")
                    for j in range(psz):
                        k = g + p0 + j
                        ls = xg[:, 0, k * P : (k + 1) * P]
                        nc.tensor.matmul(
                            hp[:, j * 512 : (j + 1) * 512], lhsT=ls, rhs=w1sb[:],
                            start=True, stop=True,
                        )
                    hts.append((p0, psz, hp))
                # leaky-relu + exp on the Act engine (Prelu & Exp share a table)
                lr4 = sp.tile([P, 8 * gsz], BF16, tag="lr")
                nc.scalar.activation(out=lr4[:], in_=eps4[:], func=AF.Prelu, alpha=NEG)
                pb4 = sp.tile([P, 8 * gsz], BF16, tag="pb")
                nc.scalar.activation(out=pb4[:], in_=lr4[:], func=AF.Exp)
                for p0, psz, hp in hts:
                    vt = vp.tile([P, psz * 8, HID], BF16, tag="vt")
                    nc.vector.tensor_tensor(
                        out=vt[:],
                        in0=hp[:].rearrange("p (h c) -> p h c", c=HID),
                        in1=pb4[:, p0 * 8 : (p0 + psz) * 8]
                        .unsqueeze(2)
                        .to_broadcast([P, psz * 8, HID]),
                        op=OP.mult,
                    )
                    for j in range(psz):
                        k = g + p0 + j
                        st = sp.tile([P, P], BF16, tag="st")
                        nc.vector.tensor_scalar(
                            out=st[:], in0=iosb[:],
                            scalar1=dlsb[:, co + k : co + k + 1],
                            scalar2=None, op0=OP.is_equal,
                        )
                        nc.tensor.matmul(
                            o1ps[:], lhsT=st[:],
                            rhs=vt[:, j * 8 : (j + 1) * 8, :].rearrange(
                                "p h c -> p (h c)"
                            ),
                            start=(k == 0), stop=(k == nchk - 1),
                        )
                        nc.tensor.matmul(
                            zps, lhsT=st[:],
                            rhs=pb4[:, (p0 + j) * 8 : (p0 + j + 1) * 8],
                            start=(k == 0), stop=(k == nchk - 1),
                        )

            zr = sp.tile([P, HEADS], F32, tag="zr")
            nc.vector.reciprocal(zr[:], zps)
            r1 = rp.tile([P, HEADS * HID], BF16, tag="r1")
            for h in range(HEADS):
                nc.scalar.activation(
                    out=r1[:, h * HID : (h + 1) * HID],
                    in_=o1ps[:, h * HID : (h + 1) * HID],
                    func=AF.Relu,
                    scale=zr[:, h : h + 1],
                )
            h2ps = zz[:, 64:106]
            for j in range(4):
                tp = pt.tile([P, P], BF16, tag="tp")
                nc.tensor.transpose(tp[:], r1[:, j * P : (j + 1) * P], idsb[:])
                tsb = rp.tile([P, P], BF16, tag="tsb")
                nc.scalar.activation(out=tsb[:], in_=tp[:], func=AF.Copy)
                nc.tensor.matmul(
                    h2ps, lhsT=tsb[:], rhs=w2csb[:, j * 42 : (j + 1) * 42],
                    start=(j == 0), stop=(j == 3),
                )
            o1 = rp.tile([P, 64], F32, tag="o1s")
            nc.scalar.activation(out=o1[:, 0:42], in_=h2ps, func=AF.Copy)
            nc.vector.memset(o1[:, 42:64], 0.0)
            nc.sync.dma_start(h2row.ap()[t * P : (t + 1) * P, :], o1[:])

            oa += ea // 16
            ob += eb // 16
            od += ept // 16
            co += nchk
    nc.compile()
    return nc


def _build_l2(EA, EB):
    colsA = sum(EA) // 16
    colsB = sum(EB) // 16
    EPT = [a + b for a, b in zip(EA, EB)]
    colsD = sum(EPT) // 16
    nch_tot = sum(EPT) // P

    colsAD = colsA + colsD
    nc = bacc.Bacc("TRN2", target_bir_lowering=False, debug=False, num_devices=NCORE)
    htA = nc.dram_tensor("htA", [SPLIT, 64], F32, kind="ExternalInput")
    htB = nc.dram_tensor("htB", [NB, 64], F32, kind="ExternalInput")
    idxAD = nc.dram_tensor("idxAD", [P, colsAD], I16, kind="ExternalInput")
    idxB = nc.dram_tensor("idxB", [P, max(colsB, 1)], I16, kind="ExternalInput")
    dloc = nc.dram_tensor("dloc", [P, nch_tot], F32, kind="ExternalInput")
    iot = nc.dram_tensor("iot", [P, P], BF16, kind="ExternalInput")
    out2 = nc.dram_tensor("out2", [SHARD, OUT], F32, kind="ExternalOutput")

    with tile.TileContext(nc) as tc, ExitStack() as ctx:
        cp = ctx.enter_context(tc.tile_pool(name="const", bufs=1))
        gp = ctx.enter_context(tc.tile_pool(name="gath", bufs=3))
        sp = ctx.enter_context(tc.tile_pool(name="small", bufs=6))
        rp = ctx.enter_context(tc.tile_pool(name="tail", bufs=2))
        po = ctx.enter_context(tc.tile_pool(name="po", bufs=2, space="PSUM"))
        pz = ctx.enter_context(tc.tile_pool(name="pz", bufs=2, space="PSUM"))

        iosb = cp.tile([P, P], BF16)
        nc.sync.dma_start(iosb[:], iot.ap())
        iADsb = cp.tile([P, colsAD], I16)
        nc.sync.dma_start(iADsb[:], idxAD.ap())
        iBsb = cp.tile([P, max(colsB, 1)], I16)
        nc.sync.dma_start(iBsb[:], idxB.ap())
        dlsb = cp.tile([P, nch_tot], F32)
        nc.sync.dma_start(dlsb[:], dloc.ap())
        # persistent per-tile stashes for the final batched log-softmax
        tmAll = cp.tile([P, TILES * OUT], F32)
        smAll = cp.tile([P, TILES], F32)

        oad = ob = co = 0
        for t in range(TILES):
            ea, eb = EA[t], EB[t]
            ept = ea + eb
            nchk = ept // P
            eaP, ebP = ea // P, eb // P
            # tile layout: [srcA chunks | dst chunks | srcB chunks]
            gad = gp.tile([P, nchk + ept // P, 64], F32, tag="g2")
            _gather(nc, gad[:, 0 : (ea + ept) // P, :], htA.ap(), iADsb, oad,
                    ea + ept, 64)
            if eb:
                _gather(nc, gad[:, (ea + ept) // P :, :], htB.ap(), iBsb, ob,
                        eb, 64)

            def _src(k, ea=ea, ept=ept, gad=gad):
                return gad[:, k, :] if k < ea // P else gad[:, ept // P + k, :]

            # batched logits for the whole tile: lg[:, k] = e2src + e2dst
            lg = sp.tile([P, nchk], F32, tag="lg")
            if eaP:
                nc.vector.tensor_tensor(
                    out=lg[:, 0:eaP].unsqueeze(2),
                    in0=gad[:, 0:eaP, 40:41],
                    in1=gad[:, eaP : 2 * eaP, 41:42],
                    op=OP.add,
                )
            if ebP:
                nc.vector.tensor_tensor(
                    out=lg[:, eaP:nchk].unsqueeze(2),
                    in0=gad[:, nchk + eaP : 2 * nchk, 40:41],
                    in1=gad[:, 2 * eaP : eaP + nchk, 41:42],
                    op=OP.add,
                )
            lr = sp.tile([P, nchk], F32, tag="lr")
            nc.vector.scalar_tensor_tensor(
                out=lr[:], in0=lg[:], scalar=NEG, in1=lg[:],
                op0=OP.mult, op1=OP.max,
            )
            pb = sp.tile([P, nchk], BF16, tag="pb")
            nc.scalar.activation(out=pb[:], in_=lr[:], func=AF.Exp)

            o2ps = po.tile([P, 48], F32, tag="o2")
            z2ps = pz.tile([P, 8], F32, tag="z2")
            for k in range(nchk):
                st = sp.tile([P, P], BF16, tag="st")
                nc.vector.tensor_scalar(
                    out=st[:], in0=iosb[:], scalar1=dlsb[:, co + k : co + k + 1],
                    scalar2=None, op0=OP.is_equal,
                )
                v2 = sp.tile([P, OUT], BF16, tag="v2")
                nc.vector.tensor_scalar(
                    out=v2[:], in0=_src(k)[:, 0:OUT], scalar1=pb[:, k : k + 1],
                    scalar2=None, op0=OP.mult,
                )
                nc.tensor.matmul(
                    o2ps[:, 0:OUT], lhsT=st[:], rhs=v2[:],
                    start=(k == 0), stop=(k == nchk - 1),
                )
                nc.tensor.matmul(
                    z2ps[:, 0:1], lhsT=st[:], rhs=pb[:, k : k + 1],
                    start=(k == 0), stop=(k == nchk - 1),
                )

            zr = sp.tile([P, 1], F32, tag="zr")
            nc.vector.reciprocal(zr[:], z2ps[:, 0:1])
            av = rp.tile([P, OUT], F32, tag="av")
            nc.vector.tensor_scalar(
                out=av[:], in0=o2ps[:, 0:OUT], scalar1=zr[:], scalar2=None,
                op0=OP.mult,
            )
            mx = sp.tile([P, 1], F32, tag="mx")
            nc.vector.reduce_max(out=mx[:], in_=av[:], axis=mybir.AxisListType.X)
            nc.vector.tensor_scalar(
                out=tmAll[:, t * OUT : (t + 1) * OUT], in0=av[:], scalar1=mx[:],
                scalar2=None, op0=OP.subtract,
            )
            ex = rp.tile([P, OUT], F32, tag="ex")
            nc.scalar.activation(
                out=ex[:], in_=tmAll[:, t * OUT : (t + 1) * OUT], func=AF.Exp,
                accum_out=smAll[:, t : t + 1],
            )

            oad += (ea + ept) // 16
            ob += eb // 16
            co += nchk

        # single Ln pass (one act-table swap), then final subtract + store
        lnA = cp.tile([P, TILES], F32)
        nc.scalar.activation(out=lnA[:], in_=smAll[:], func=AF.Ln)
        for t in range(TILES):
            fin = rp.tile([P, OUT], F32, tag="fin")
            nc.vector.tensor_scalar(
                out=fin[:], in0=tmAll[:, t * OUT : (t + 1) * OUT],
                scalar1=lnA[:, t : t + 1], scalar2=None, op0=OP.subtract,
            )
            nc.sync.dma_start(out2.ap()[t * P : (t + 1) * P, :], fin[:])
    nc.compile()
    return nc


def _prepare(x, edge_index, W1, a1_src, a1_dst, W2, a2_src, a2_dst):
    key = hash(np.asarray(edge_index).tobytes())
    if key in _CACHE:
        return _CACHE[key]
    EA, EB, streams = _prep_edges(edge_index)
    l1 = _build_l1(EA, EB)
    l2 = _build_l2(EA, EB)
    _CACHE.clear()
    _CACHE[key] = (EA, EB, streams, l1, l2)
    return _CACHE[key]


def _host_consts(x, W1, a1_src, a1_dst, W2, a2_src, a2_dst):
    x = np.asarray(x, np.float32)
    W1 = np.asarray(W1, np.float32)
    W2 = np.asarray(W2, np.float32)
    a1_src = np.asarray(a1_src, np.float32)
    a1_dst = np.asarray(a1_dst, np.float32)
    a2_src = np.asarray(a2_src, np.float32).reshape(-1)
    a2_dst = np.asarray(a2_dst, np.float32).reshape(-1)

    xpad = np.zeros((NPAD, IN), np.float32)
    xpad[:N] = x
    W1r = W1.reshape(IN, HEADS, HID)
    wsd = np.concatenate(
        [np.einsum("khc,hc->kh", W1r, a1_src), np.einsum("khc,hc->kh", W1r, a1_dst)],
        axis=1,
    )  # [128, 16]
    wv2s = W2 @ a2_src  # [512]
    wv2d = W2 @ a2_dst
    w2c = np.zeros((P, 4 * 42), np.float32)
    for j in range(4):
        w2c[:, j * 42 : j * 42 + 40] = W2[j * P : (j + 1) * P, :]
        w2c[:, j * 42 + 40] = wv2s[j * P : (j + 1) * P]
        w2c[:, j * 42 + 41] = wv2d[j * P : (j + 1) * P]
    iot = np.tile(np.arange(P, dtype=np.float32), (P, 1)).astype(_bf16)
    idn = np.eye(P, dtype=np.float32)
    return xpad, wsd.astype(_bf16), w2c.astype(_bf16), iot, idn.astype(_bf16), W1.astype(_bf16)


def _run(inputs, trace=False):
    x = inputs["x"]
    edge_index = inputs["edge_index"]
    EA, EB, streams, l1, l2 = _prepare(
        x, edge_index, inputs["W1"], inputs["a1_src"], inputs["a1_dst"],
        inputs["W2"], inputs["a2_src"], inputs["a2_dst"],
    )
    xpad, wsd, w2c, iot, idn, W1bf = _host_consts(
        x, inputs["W1"], inputs["a1_src"], inputs["a1_dst"],
        inputs["W2"], inputs["a2_src"], inputs["a2_dst"],
    )

    in_maps = []
    for c in range(NCORE):
        xr = np.roll(xpad, -c * SHARD, axis=0).astype(_bf16)
        s = streams[c]
        in_maps.append(
            dict(
                xtA=xr[:SPLIT], xtB=xr[SPLIT:],
                idxA=s["idxA"], idxB=s["idxB"], idxD=s["idxD"],
                dloc=np.ascontiguousarray(s["dloc"]),
                w1=W1bf, wsd=wsd, w2c=w2c, iot=iot, idn=idn,
            )
        )
    def _launch(prog, maps):
        try:
            return run_bass_kernel_spmd(prog, maps, list(range(NCORE)), trace=trace)
        except Exception:
            import time as _time
            _time.sleep(5)
            return run_bass_kernel_spmd(prog, maps, list(range(NCORE)), trace=trace)

    r1 = _launch(l1, in_maps)
    h2tab = np.zeros((NPAD, 64), np.float32)
    for c in range(NCORE):
        h2tab[c * SHARD : (c + 1) * SHARD] = r1.results[c]["h2row"]
    h2tab[N:] = 0.0

    in_maps2 = []
    for c in range(NCORE):
        hr = np.roll(h2tab, -c * SHARD, axis=0)
        s = streams[c]
        in_maps2.append(
            dict(
                htA=np.ascontiguousarray(hr[:SPLIT]),
                htB=np.ascontiguousarray(hr[SPLIT:]),
                idxAD=s["idxAD"], idxB=s["idxB"],
                dloc=np.ascontiguousarray(s["dloc"]), iot=iot,
            )
        )
    r2 = _launch(l2, in_maps2)
    out = np.concatenate([r2.results[c]["out2"] for c in range(NCORE)], axis=0)[:N]
    ns = None
    if r1.exec_time_ns is not None and r2.exec_time_ns is not None:
        ns = r1.exec_time_ns + r2.exec_time_ns
    return np.ascontiguousarray(out, dtype=np.float32), ns


def kernel(**inputs) -> np.ndarray:
    out, _ = _run(inputs, trace=False)
    return out



# revision 11
# speedup vs baseline: 1.2566x; 1.0026x over previous
"""GAT (2-layer, PyG-style) on 8 Trainium2 NeuronCores.

Strategy: destination-node sharding (graph parallel). Each core owns a
contiguous range of 6272 destination nodes and all edges pointing into
them (sorted by dst). Source-node features are fetched on-device with
batched indexed gathers (dma_gather) from a per-core *rotated* copy of
the node table, so that a core's own dst rows always sit at small row
indices (dma_gather indices are int16, hence also the A/B table-half
split for the random source indices).

Layer-1 messages are computed on the fly: gather x[src] (bf16, 256 B
rows), then h = x@W1 / e_src = x@w_src / e_dst = x@w_dst as PE matmuls
per 128-edge chunk; segment softmax + scatter-add are done with a
one-hot matmul (S_T^T @ V) accumulated in PSUM per 128-dst-node tile.
The tile tail normalizes by the softmax denominator, applies ReLU and
immediately computes the layer-2 node table row [h2 | e2_src | e2_dst]
via W2. A second launch runs the (structurally identical, 1-head)
layer-2 attention over the host-assembled h2 table and finishes with
log_softmax.
"""

import numpy as np
import ml_dtypes
from contextlib import ExitStack

import concourse.bass as bass
import concourse.mybir as mybir
import concourse.tile as tile
from concourse import bacc
from concourse.bass_utils import run_bass_kernel_spmd

F32 = mybir.dt.float32
BF16 = mybir.dt.bfloat16
I16 = mybir.dt.int16
AF = mybir.ActivationFunctionType
OP = mybir.AluOpType

N = 50000
E = 500000
IN = 128
HID = 64
HEADS = 8
OUT = 40
NEG = 0.2
NCORE = 8
P = 128
TILES = 49
SHARD = TILES * P          # 6272
NPAD = NCORE * SHARD       # 50176
SPLIT = 32768              # int16 table-half split
NB = NPAD - SPLIT          # 17408

_bf16 = ml_dtypes.bfloat16

_CACHE = {}

# Gather sizing: one dma_gather of n indices emits n/16+2 descriptors per
# SDMA engine; with single_packet=True a packet holds at most 64
# descriptors, so calls >992 indices wedge the device. 896 indices
# (58 descs) stays under the cap; multi-call concurrency at this size is
# throttled safely by ucode (verified on HW).
GCAP = 896
GSP = True  # single_packet


def _gather(nc, out3, in_ap, idx_sb, col0, n, elem):
    """dma_gather split into <=GCAP-index calls. out3: [P, 1|chunks, *]
    destination AP covering exactly n indices starting at its origin."""
    done = 0
    while done < n:
        take = min(GCAP, n - done)
        if out3.ndim == 3 and out3.shape[2] != elem:  # transpose=True layout
            o = out3[:, :, done : done + take]
            tr = True
        else:  # [P, chunks, elem] layout
            o = out3[:, done // P : (done + take) // P, :]
            tr = False
        nc.gpsimd.dma_gather(
            out_ap=o,
            in_ap=in_ap,
            idxs_ap=idx_sb[:, col0 + done // 16 : col0 + (done + take) // 16],
            num_idxs=take,
            num_idxs_reg=take,
            elem_size=elem,
            transpose=tr,
            single_packet=GSP,
        )
        done += take


def _wrap16(v):
    """dma_gather index layout: idx[p, j] = stream[j*16 + p%16], replicated
    to 128 partitions."""
    assert len(v) % 16 == 0
    w = v.reshape(-1, 16).T.astype(np.int16)   # [16, n/16]
    return np.tile(w, (8, 1))                  # [128, n/16]


def _prep_edges(edge_index):
    """Bucket edges (+self-loops) by dst core, sort by dst, split by
    src-table half, pad to SPMD-uniform per-tile sizes.

    Returns per-tile padded sizes EA/EB (shared by all cores) and the
    per-core index/metadata streams."""
    src = np.concatenate([np.asarray(edge_index[0]), np.arange(N)]).astype(np.int64)
    dst = np.concatenate([np.asarray(edge_index[1]), np.arange(N)]).astype(np.int64)
    core = dst // SHARD

    pc = []  # per-core (tile -> (a_idx, b_idx, dloc_a, dloc_b))
    nA = np.zeros((NCORE, TILES), np.int64)
    nB = np.zeros((NCORE, TILES), np.int64)
    for c in range(NCORE):
        m = core == c
        s = src[m]
        dl = dst[m] - c * SHARD
        o = np.argsort(dl, kind="stable")
        s = s[o]
        dl = dl[o]
        sr = (s - c * SHARD) % NPAD  # rotated source row
        bounds = np.searchsorted(dl, np.arange(TILES + 1) * P)
        tl = []
        for t in range(TILES):
            lo, hi = bounds[t], bounds[t + 1]
            srt, dlt = sr[lo:hi], dl[lo:hi] % P
            ma = srt < SPLIT
            tl.append((srt[ma], srt[~ma] - SPLIT, dlt[ma], dlt[~ma]))
            nA[c, t] = ma.sum()
            nB[c, t] = (~ma).sum()
        pc.append(tl)

    rup = lambda n: int(-(-n // P) * P)
    EA = [rup(nA[:, t].max()) for t in range(TILES)]
    EB = [rup(nB[:, t].max()) for t in range(TILES)]

    streams = []
    for c in range(NCORE):
        ia, ib, idd, dlc = [], [], [], []
        for t in range(TILES):
            a, b, da, db = pc[c][t]
            pa = np.zeros(EA[t], np.int64)
            pa[: len(a)] = a
            pb = np.zeros(EB[t], np.int64)
            pb[: len(b)] = b
            ia.append(pa)
            ib.append(pb)
            # dst-row gather stream + dst-local values, in slot order [A|B]
            dr = np.zeros(EA[t] + EB[t], np.int64)
            dv = np.full(EA[t] + EB[t], 200.0, np.float32)
            dr[: len(a)] = da + t * P
            dv[: len(a)] = da
            dr[EA[t] : EA[t] + len(b)] = db + t * P
            dv[EA[t] : EA[t] + len(b)] = db
            idd.append(dr)
            dlc.append(dv)
        ept = np.concatenate(idd)
        # L2 combined stream: per tile [srcA-padded | dst] (both read htA)
        iad = [np.concatenate([ia[t], idd[t]]) for t in range(TILES)]
        streams.append(
            dict(
                idxA=_wrap16(np.concatenate(ia)),
                idxB=_wrap16(np.concatenate(ib)),
                idxD=_wrap16(ept),
                idxAD=_wrap16(np.concatenate(iad)),
                dloc=np.concatenate(dlc).reshape(-1, P).T.copy(),  # [128, nchunks]
            )
        )
    return EA, EB, streams


def _build_l1(EA, EB):
    colsA = sum(EA) // 16
    colsB = sum(EB) // 16
    EPT = [a + b for a, b in zip(EA, EB)]
    colsD = sum(EPT) // 16
    nch_tot = sum(EPT) // P

    nc = bacc.Bacc("TRN2", target_bir_lowering=False, debug=False, num_devices=NCORE)
    xtA = nc.dram_tensor("xtA", [SPLIT, IN], BF16, kind="ExternalInput")
    xtB = nc.dram_tensor("xtB", [NB, IN], BF16, kind="ExternalInput")
    idxA = nc.dram_tensor("idxA", [P, max(colsA, 1)], I16, kind="ExternalInput")
    idxB = nc.dram_tensor("idxB", [P, max(colsB, 1)], I16, kind="ExternalInput")
    idxD = nc.dram_tensor("idxD", [P, colsD], I16, kind="ExternalInput")
    dloc = nc.dram_tensor("dloc", [P, nch_tot], F32, kind="ExternalInput")
    w1 = nc.dram_tensor("w1", [P, HEADS * HID], BF16, kind="ExternalInput")
    wsd = nc.dram_tensor("wsd", [P, 2 * HEADS], BF16, kind="ExternalInput")
    w2c = nc.dram_tensor("w2c", [P, 4 * 42], BF16, kind="ExternalInput")
    iot = nc.dram_tensor("iot", [P, P], BF16, kind="ExternalInput")
    idn = nc.dram_tensor("idn", [P, P], BF16, kind="ExternalInput")
    h2row = nc.dram_tensor("h2row", [SHARD, 64], F32, kind="ExternalOutput")

    with tile.TileContext(nc) as tc, ExitStack() as ctx:
        cp = ctx.enter_context(tc.tile_pool(name="const", bufs=1))
        gp = ctx.enter_context(tc.tile_pool(name="gath", bufs=12))
        sp = ctx.enter_context(tc.tile_pool(name="small", bufs=12))
        vp = ctx.enter_context(tc.tile_pool(name="vals", bufs=4))
        rp = ctx.enter_context(tc.tile_pool(name="tail", bufs=3))
        # PSUM budget (8 banks): ph h-pairs 2x2=4, pm eps 1, pt tp 1, po o1 1,
        # pz 1 (z cols 0:8 + h2 cols 64:112 carved from one bank tile)
        ph = ctx.enter_context(tc.tile_pool(name="ph", bufs=2, space="PSUM"))
        pm = ctx.enter_context(tc.tile_pool(name="pm", bufs=1, space="PSUM"))
        pt = ctx.enter_context(tc.tile_pool(name="pt", bufs=1, space="PSUM"))
        po = ctx.enter_context(tc.tile_pool(name="po", bufs=1, space="PSUM"))
        pz = ctx.enter_context(tc.tile_pool(name="pz", bufs=1, space="PSUM"))

        w1sb = cp.tile([P, HEADS * HID], BF16)
        nc.sync.dma_start(w1sb[:], w1.ap())
        wsdsb = cp.tile([P, 2 * HEADS], BF16)
        nc.sync.dma_start(wsdsb[:], wsd.ap())
        w2csb = cp.tile([P, 4 * 42], BF16)
        nc.sync.dma_start(w2csb[:], w2c.ap())
        iosb = cp.tile([P, P], BF16)
        nc.sync.dma_start(iosb[:], iot.ap())
        idsb = cp.tile([P, P], BF16)
        nc.sync.dma_start(idsb[:], idn.ap())
        iAsb = cp.tile([P, max(colsA, 1)], I16)
        nc.sync.dma_start(iAsb[:], idxA.ap())
        iBsb = cp.tile([P, max(colsB, 1)], I16)
        nc.sync.dma_start(iBsb[:], idxB.ap())
        iDsb = cp.tile([P, colsD], I16)
        nc.sync.dma_start(iDsb[:], idxD.ap())
        dlsb = cp.tile([P, nch_tot], F32)
        nc.sync.dma_start(dlsb[:], dloc.ap())

        oa = ob = od = co = 0
        for t in range(TILES):
            ea, eb = EA[t], EB[t]
            ept = ea + eb
            nchk = ept // P
            xg = gp.tile([P, 1, ept], BF16, tag="xg")
            if ea:
                _gather(nc, xg[:, :, 0:ea], xtA.ap(), iAsb, oa, ea, IN)
            if eb:
                _gather(nc, xg[:, :, ea:ept], xtB.ap(), iBsb, ob, eb, IN)
            xd = gp.tile([P, 1, ept], BF16, tag="xd")
            _gather(nc, xd[:, :, 0:ept], xtA.ap(), iDsb, od, ept, IN)

            o1ps = po.tile([P, HEADS * HID], F32, tag="o1")
            zz = pz.tile([P, 512], F32, tag="zz")
            zps = zz[:, 0:HEADS]
            G = 4
            for g in range(0, nchk, G):
                gsz = min(G, nchk - g)
                # logits for the group: one PSUM tile, one 8-col region/chunk
                eps4 = pm.tile([P, 8 * gsz], F32, tag="eps")
                for j in range(gsz):
                    ls = xg[:, 0, (g + j) * P : (g + j + 1) * P]
                    ld = xd[:, 0, (g + j) * P : (g + j + 1) * P]
                    nc.tensor.matmul(
                        eps4[:, j * 8 : (j + 1) * 8], lhsT=ls,
                        rhs=wsdsb[:, 0:HEADS], start=True, stop=False,
                    )
                    nc.tensor.matmul(
                        eps4[:, j * 8 : (j + 1) * 8], lhsT=ld,
                        rhs=wsdsb[:, HEADS : 2 * HEADS], start=False, stop=True,
                    )
                # h for the group: chunk pairs share a 2-bank PSUM tile
                hts = []
                for p0 in range(0, gsz, 2):
                    psz = min(2, gsz - p0)
                    hp = ph.tile([P, psz * 512], F32, tag="h")
                    for j in range(psz):
                        k = g + p0 + j
                        ls = xg[:, 0, k * P : (k + 1) * P]
                        nc.tensor.matmul(
                            hp[:, j * 512 : (j + 1) * 512], lhsT=ls, rhs=w1sb[:],
                            start=True, stop=True,
                        )
                    hts.append((p0, psz, hp))
                # leaky-relu + exp on the Act engine (Prelu & Exp share a table)
                lr4 = sp.tile([P, 8 * gsz], BF16, tag="lr")
                nc.scalar.activation(out=lr4[:], in_=eps4[:], func=AF.Prelu, alpha=NEG)
                pb4 = sp.tile([P, 8 * gsz], BF16, tag="pb")
                nc.scalar.activation(out=pb4[:], in_=lr4[:], func=AF.Exp)
                for p0, psz, hp in hts:
                    vt = vp.tile([P, psz * 8, HID], BF16, tag="vt")
                    nc.vector.tensor_tensor(
                        out=vt[:],
                        in0=hp[:].rearrange("p (h c) -> p h c", c=HID),
                        in1=pb4[:, p0 * 8 : (p0 + psz) * 8]
                        .unsqueeze(2)
                        .to_broadcast([P, psz * 8, HID]),
                        op=OP.mult,
                    )
                    for j in range(psz):
                        k = g + p0 + j
                        st = sp.tile([P, P], BF16, tag="st")
                        nc.vector.tensor_scalar(
                            out=st[:], in0=iosb[:],
                            scalar1=dlsb[:, co + k : co + k + 1],
                            scalar2=None, op0=OP.is_equal,
                        )
                        nc.tensor.matmul(
                            o1ps[:], lhsT=st[:],
                            rhs=vt[:, j * 8 : (j + 1) * 8, :].rearrange(
                                "p h c -> p (h c)"
                            ),
                            start=(k == 0), stop=(k == nchk - 1),
                        )
                        nc.tensor.matmul(
                            zps, lhsT=st[:],
                            rhs=pb4[:, (p0 + j) * 8 : (p0 + j + 1) * 8],
                            start=(k == 0), stop=(k == nchk - 1),
                        )

            zr = sp.tile([P, HEADS], F32, tag="zr")
            nc.vector.reciprocal(zr[:], zps)
            r1 = rp.tile([P, HEADS * HID], BF16, tag="r1")
            for h in range(HEADS):
                nc.scalar.activation(
                    out=r1[:, h * HID : (h + 1) * HID],
                    in_=o1ps[:, h * HID : (h + 1) * HID],
                    func=AF.Relu,
                    scale=zr[:, h : h + 1],
                )
            h2ps = zz[:, 64:106]
            for j in range(4):
                tp = pt.tile([P, P], BF16, tag="tp")
                nc.tensor.transpose(tp[:], r1[:, j * P : (j + 1) * P], idsb[:])
                tsb = rp.tile([P, P], BF16, tag="tsb")
                nc.scalar.activation(out=tsb[:], in_=tp[:], func=AF.Copy)
                nc.tensor.matmul(
                    h2ps, lhsT=tsb[:], rhs=w2csb[:, j * 42 : (j + 1) * 42],
                    start=(j == 0), stop=(j == 3),
                )
            o1 = rp.tile([P, 64], F32, tag="o1s")
            nc.scalar.activation(out=o1[:, 0:42], in_=h2ps, func=AF.Copy)
            nc.vector.memset(o1[:, 42:64], 0.0)
            nc.sync.dma_start(h2row.ap()[t * P : (t + 1) * P, :], o1[:])

            oa += ea // 16
            ob += eb // 16
            od += ept // 16
            co += nchk
    nc.compile()
    return nc


def _build_l2(EA, EB):
    colsA = sum(EA) // 16
    colsB = sum(EB) // 16
    EPT = [a + b for a, b in zip(EA, EB)]
    colsD = sum(EPT) // 16
    nch_tot = sum(EPT) // P

    colsAD = colsA + colsD
    nc = bacc.Bacc("TRN2", target_bir_lowering=False, debug=False, num_devices=NCORE)
    htA = nc.dram_tensor("htA", [SPLIT, 64], F32, kind="ExternalInput")
    htB = nc.dram_tensor("htB", [NB, 64], F32, kind="ExternalInput")
    idxAD = nc.dram_tensor("idxAD", [P, colsAD], I16, kind="ExternalInput")
    idxB = nc.dram_tensor("idxB", [P, max(colsB, 1)], I16, kind="ExternalInput")
    dloc = nc.dram_tensor("dloc", [P, nch_tot], F32, kind="ExternalInput")
    iot = nc.dram_tensor("iot", [P, P], BF16, kind="ExternalInput")
    out2 = nc.dram_tensor("out2", [SHARD, OUT], F32, kind="ExternalOutput")

    with tile.TileContext(nc) as tc, ExitStack() as ctx:
        cp = ctx.enter_context(tc.tile_pool(name="const", bufs=1))
        gp = ctx.enter_context(tc.tile_pool(name="gath", bufs=3))
        sp = ctx.enter_context(tc.tile_pool(name="small", bufs=6))
        rp = ctx.enter_context(tc.tile_pool(name="tail", bufs=2))
        po = ctx.enter_context(tc.tile_pool(name="po", bufs=2, space="PSUM"))
        pz = ctx.enter_context(tc.tile_pool(name="pz", bufs=2, space="PSUM"))

        iosb = cp.tile([P, P], BF16)
        nc.sync.dma_start(iosb[:], iot.ap())
        iADsb = cp.tile([P, colsAD], I16)
        nc.sync.dma_start(iADsb[:], idxAD.ap())
        iBsb = cp.tile([P, max(colsB, 1)], I16)
        nc.sync.dma_start(iBsb[:], idxB.ap())
        dlsb = cp.tile([P, nch_tot], F32)
        nc.sync.dma_start(dlsb[:], dloc.ap())
        # persistent per-tile stashes for the final batched log-softmax
        tmAll = cp.tile([P, TILES * OUT], F32)
        smAll = cp.tile([P, TILES], F32)

        oad = ob = co = 0
        for t in range(TILES):
            ea, eb = EA[t], EB[t]
            ept = ea + eb
            nchk = ept // P
            eaP, ebP = ea // P, eb // P
            # tile layout: [srcA chunks | dst chunks | srcB chunks]
            gad = gp.tile([P, nchk + ept // P, 64], F32, tag="g2")
            _gather(nc, gad[:, 0 : (ea + ept) // P, :], htA.ap(), iADsb, oad,
                    ea + ept, 64)
            if eb:
                _gather(nc, gad[:, (ea + ept) // P :, :], htB.ap(), iBsb, ob,
                        eb, 64)

            def _src(k, ea=ea, ept=ept, gad=gad):
                return gad[:, k, :] if k < ea // P else gad[:, ept // P + k, :]

            # batched logits for the whole tile: lg[:, k] = e2src + e2dst
            lg = sp.tile([P, nchk], F32, tag="lg")
            if eaP:
                nc.vector.tensor_tensor(
                    out=lg[:, 0:eaP].unsqueeze(2),
                    in0=gad[:, 0:eaP, 40:41],
                    in1=gad[:, eaP : 2 * eaP, 41:42],
                    op=OP.add,
                )
            if ebP:
                nc.vector.tensor_tensor(
                    out=lg[:, eaP:nchk].unsqueeze(2),
                    in0=gad[:, nchk + eaP : 2 * nchk, 40:41],
                    in1=gad[:, 2 * eaP : eaP + nchk, 41:42],
                    op=OP.add,
                )
            lr = sp.tile([P, nchk], F32, tag="lr")
            nc.vector.scalar_tensor_tensor(
                out=lr[:], in0=lg[:], scalar=NEG, in1=lg[:],
                op0=OP.mult, op1=OP.max,
            )
            pb = sp.tile([P, nchk], F32, tag="pb")
            nc.scalar.activation(out=pb[:], in_=lr[:], func=AF.Exp)
            pbb = sp.tile([P, nchk], BF16, tag="pbb")
            nc.vector.tensor_copy(out=pbb[:], in_=pb[:])

            o2ps = po.tile([P, 48], F32, tag="o2")
            z2ps = pz.tile([P, 8], F32, tag="z2")
            for k in range(nchk):
                st = sp.tile([P, P], BF16, tag="st")
                nc.vector.tensor_scalar(
                    out=st[:], in0=iosb[:], scalar1=dlsb[:, co + k : co + k + 1],
                    scalar2=None, op0=OP.is_equal,
                )
                v2 = sp.tile([P, OUT], BF16, tag="v2")
                nc.vector.tensor_scalar(
                    out=v2[:], in0=_src(k)[:, 0:OUT], scalar1=pb[:, k : k + 1],
                    scalar2=None, op0=OP.mult,
                )
                nc.tensor.matmul(
                    o2ps[:, 0:OUT], lhsT=st[:], rhs=v2[:],
                    start=(k == 0), stop=(k == nchk - 1),
                )
                nc.tensor.matmul(
                    z2ps[:, 0:1], lhsT=st[:], rhs=pbb[:, k : k + 1],
                    start=(k == 0), stop=(k == nchk - 1),
                )

            zr = sp.tile([P, 1], F32, tag="zr")
            nc.vector.reciprocal(zr[:], z2ps[:, 0:1])
            av = rp.tile([P, OUT], F32, tag="av")
            nc.vector.tensor_scalar(
                out=av[:], in0=o2ps[:, 0:OUT], scalar1=zr[:], scalar2=None,
                op0=OP.mult,
            )
            mx = sp.tile([P, 1], F32, tag="mx")
            nc.vector.reduce_max(out=mx[:], in_=av[:], axis=mybir.AxisListType.X)
            nc.vector.tensor_scalar(
                out=tmAll[:, t * OUT : (t + 1) * OUT], in0=av[:], scalar1=mx[:],
                scalar2=None, op0=OP.subtract,
            )
            ex = rp.tile([P, OUT], F32, tag="ex")
            nc.scalar.activation(
                out=ex[:], in_=tmAll[:, t * OUT : (t + 1) * OUT], func=AF.Exp,
                accum_out=smAll[:, t : t + 1],
            )

            oad += (ea + ept) // 16
            ob += eb // 16
            co += nchk

        # single Ln pass (one act-table swap), then final subtract + store
        lnA = cp.tile([P, TILES], F32)
        nc.scalar.activation(out=lnA[:], in_=smAll[:], func=AF.Ln)
        for t in range(TILES):
            fin = rp.tile([P, OUT], F32, tag="fin")
            nc.vector.tensor_scalar(
                out=fin[:], in0=tmAll[:, t * OUT : (t + 1) * OUT],
                scalar1=lnA[:, t : t + 1], scalar2=None, op0=OP.subtract,
            )
            nc.sync.dma_start(out2.ap()[t * P : (t + 1) * P, :], fin[:])
    nc.compile()
    return nc


def _prepare(x, edge_index, W1, a1_src, a1_dst, W2, a2_src, a2_dst):
    key = hash(np.asarray(edge_index).tobytes())
    if key in _CACHE:
        return _CACHE[key]
    EA, EB, streams = _prep_edges(edge_index)
    l1 = _build_l1(EA, EB)
    l2 = _build_l2(EA, EB)
    _CACHE.clear()
    _CACHE[key] = (EA, EB, streams, l1, l2)
    return _CACHE[key]


def _host_consts(x, W1, a1_src, a1_dst, W2, a2_src, a2_dst):
    x = np.asarray(x, np.float32)
    W1 = np.asarray(W1, np.float32)
    W2 = np.asarray(W2, np.float32)
    a1_src = np.asarray(a1_src, np.float32)
    a1_dst = np.asarray(a1_dst, np.float32)
    a2_src = np.asarray(a2_src, np.float32).reshape(-1)
    a2_dst = np.asarray(a2_dst, np.float32).reshape(-1)

    xpad = np.zeros((NPAD, IN), np.float32)
    xpad[:N] = x
    W1r = W1.reshape(IN, HEADS, HID)
    wsd = np.concatenate(
        [np.einsum("khc,hc->kh", W1r, a1_src), np.einsum("khc,hc->kh", W1r, a1_dst)],
        axis=1,
    )  # [128, 16]
    wv2s = W2 @ a2_src  # [512]
    wv2d = W2 @ a2_dst
    w2c = np.zeros((P, 4 * 42), np.float32)
    for j in range(4):
        w2c[:, j * 42 : j * 42 + 40] = W2[j * P : (j + 1) * P, :]
        w2c[:, j * 42 + 40] = wv2s[j * P : (j + 1) * P]
        w2c[:, j * 42 + 41] = wv2d[j * P : (j + 1) * P]
    iot = np.tile(np.arange(P, dtype=np.float32), (P, 1)).astype(_bf16)
    idn = np.eye(P, dtype=np.float32)
    return xpad, wsd.astype(_bf16), w2c.astype(_bf16), iot, idn.astype(_bf16), W1.astype(_bf16)


def _run(inputs, trace=False):
    x = inputs["x"]
    edge_index = inputs["edge_index"]
    EA, EB, streams, l1, l2 = _prepare(
        x, edge_index, inputs["W1"], inputs["a1_src"], inputs["a1_dst"],
        inputs["W2"], inputs["a2_src"], inputs["a2_dst"],
    )
    xpad, wsd, w2c, iot, idn, W1bf = _host_consts(
        x, inputs["W1"], inputs["a1_src"], inputs["a1_dst"],
        inputs["W2"], inputs["a2_src"], inputs["a2_dst"],
    )

    in_maps = []
    for c in range(NCORE):
        xr = np.roll(xpad, -c * SHARD, axis=0).astype(_bf16)
        s = streams[c]
        in_maps.append(
            dict(
                xtA=xr[:SPLIT], xtB=xr[SPLIT:],
                idxA=s["idxA"], idxB=s["idxB"], idxD=s["idxD"],
                dloc=np.ascontiguousarray(s["dloc"]),
                w1=W1bf, wsd=wsd, w2c=w2c, iot=iot, idn=idn,
            )
        )
    def _launch(prog, maps):
        try:
            return run_bass_kernel_spmd(prog, maps, list(range(NCORE)), trace=trace)
        except Exception:
            import time as _time
            _time.sleep(5)
            return run_bass_kernel_spmd(prog, maps, list(range(NCORE)), trace=trace)

    r1 = _launch(l1, in_maps)
    h2tab = np.zeros((NPAD, 64), np.float32)
    for c in range(NCORE):
        h2tab[c * SHARD : (c + 1) * SHARD] = r1.results[c]["h2row"]
    h2tab[N:] = 0.0

    in_maps2 = []
    for c in range(NCORE):
        hr = np.roll(h2tab, -c * SHARD, axis=0)
        s = streams[c]
        in_maps2.append(
            dict(
                htA=np.ascontiguousarray(hr[:SPLIT]),
                htB=np.ascontiguousarray(hr[SPLIT:]),
                idxAD=s["idxAD"], idxB=s["idxB"],
                dloc=np.ascontiguousarray(s["dloc"]), iot=iot,
            )
        )
    r2 = _launch(l2, in_maps2)
    out = np.concatenate([r2.results[c]["out2"] for c in range(NCORE)], axis=0)[:N]
    ns = None
    if r1.exec_time_ns is not None and r2.exec_time_ns is not None:
        ns = r1.exec_time_ns + r2.exec_time_ns
    return np.ascontiguousarray(out, dtype=np.float32), ns


def kernel(**inputs) -> np.ndarray:
    out, _ = _run(inputs, trace=False)
    return out

